# revision 54
# baseline (speedup 1.0000x reference)
"""Causal self-attention (B=4, T=2048, C=1024, H=16) on 8 Trainium2 cores.

Sharding: core c -> batch b = c//2, head-group g = c%2 (8 local heads,
tensor-parallel). Partial c_proj outputs of a pair are combined with one
ReduceScatter per 512-row window; host reassembles.

Schedule is window-major (4 windows of 512 query rows) so the Scalar
engine's exp stream - the hard resource floor (~16.8M elements/core at
1 elem/cycle/lane) - starts as soon as the first QKV window is done and
runs continuously. Per window and head-pair-pair (j-pair), score matmuls
for 4 head-slots land in one 4-bank PSUM tile and a single N=2048 exp
evacuates them (minimizes ACT per-instruction overhead).

AV matmuls run "flipped": stationary = [v | ones] (M=128, FWL-friendly),
moving = p (N=512 bf16->fp8 stream). Output rows 0-63 are y^T for the
head, rows 64-127 are the softmax denominator replicated 64x (the ones
columns), so no PE transpose and no cross-partition reduction is needed.
The denominator reciprocal is computed by DVE, shifted to the opposite
partition half by a small SBUF->SBUF DMA, and fused into the PSUM->SBUF
evacuation multiply that writes y^T.

p is stored as fp8e4 (e4m3) to fit SBUF; scores/weights stay bf16.

Self-contained: only imports concourse (installed library) + numpy.
"""

import ml_dtypes
import numpy as np

import concourse.bass_utils as _bass_utils
import concourse.mybir as mybir
import concourse.tile as tile
from concourse import bacc
from concourse.bass_utils import run_bass_kernel_spmd

del _bass_utils

B, T, C = 4, 2048, 1024
H_TOTAL, D = 16, 64
N_CORES = 8
HL = H_TOTAL // 2  # local heads per core (8)
HC = HL * D  # local head cols (512)
NP = HL // 2  # head pairs (4)
P = 128
CK = C // P  # 8 contraction chunks for qkv
WIN = 512
NW = T // WIN  # 4 query windows of 512
TT = T // P  # 16 key blocks of 128
F32 = mybir.dt.float32
BF16 = mybir.dt.bfloat16
FP8 = mybir.dt.float8e4
SCALE = 1.0 / 8.0  # 1/sqrt(D)

_CACHE = {}


def _build_nc():
    nc = bacc.Bacc("TRN2", target_bir_lowering=False, debug=False, num_devices=N_CORES)

    xT_d = nc.dram_tensor("xT", [NW, P, CK, WIN], BF16, kind="ExternalInput")
    wq_d = nc.dram_tensor("wq", [P, NP, CK, P], BF16, kind="ExternalInput")
    wk_d = nc.dram_tensor("wk", [P, NP, CK, P], BF16, kind="ExternalInput")
    wv_d = nc.dram_tensor("wv", [P, CK, HC], BF16, kind="ExternalInput")
    bq_d = nc.dram_tensor("bq", [P, NP], F32, kind="ExternalInput")
    bk_d = nc.dram_tensor("bk", [P, NP], F32, kind="ExternalInput")
    bv_d = nc.dram_tensor("bv", [P, HC], BF16, kind="ExternalInput")
    wp_d = nc.dram_tensor("wp", [P, HC // P, C], BF16, kind="ExternalInput")
    bp_d = nc.dram_tensor("bp", [P, C], BF16, kind="ExternalInput")
    out_d = nc.dram_tensor("out", [T // 2, C], BF16, kind="ExternalOutput")

    with tile.TileContext(nc) as tc:
        with (
            tc.tile_pool(name="const", bufs=1) as constp,
            tc.tile_pool(name="big", bufs=1) as bigp,
            tc.tile_pool(name="xp", bufs=2) as xp,
            tc.tile_pool(name="qtp", bufs=2) as qtp,
            tc.tile_pool(name="ytp", bufs=2) as ytp,
            tc.tile_pool(name="rp", bufs=2) as rp,
            tc.tile_pool(name="nwp", bufs=1) as nwp,
            tc.tile_pool(name="zout", bufs=1) as zoutp,
            tc.tile_pool(name="score_ps", bufs=2, space="PSUM") as score_ps,
            tc.tile_pool(name="av_ps", bufs=2, space="PSUM") as av_ps,
            tc.tile_pool(name="mm_ps", bufs=2, space="PSUM") as mm_ps,
            tc.tile_pool(name="dram", bufs=1, space="DRAM") as dramp,
        ):
            # ---- constants ----
            # trimask[k, q] = 1 where q >= k else 0 (built first: the gpsimd
            # queue is about to be loaded with weight-chunk DMA triggers)
            trif = constp.tile([P, P], F32)
            nc.gpsimd.memset(trif, 1.0)
            nc.gpsimd.affine_select(
                out=trif,
                in_=trif,
                compare_op=mybir.AluOpType.is_ge,
                fill=0.0,
                base=0,
                pattern=[[1, P]],
                channel_multiplier=-1,
            )
            trimask = constp.tile([P, P], BF16)
            nc.vector.tensor_copy(out=trimask[:], in_=trif[:])
            bq_sb = constp.tile([P, NP], F32)
            bk_sb = constp.tile([P, NP], F32)
            bv_sb = constp.tile([P, HL, D], BF16)
            bp_sb = constp.tile([P, C], BF16)

            # ---- persistent tensors ----
            kT = bigp.tile([P, NP, T], BF16)  # k^T [kcol, t] (all key windows)
            # v_ext[k, kb, 2j+h, :]: h=0 -> [v(0:64) | ones], h=1 -> [ones | v(64:128)]
            # fp8 so AV matmuls can run DoubleRow (2 key blocks per matmul)
            v_ext = bigp.tile([P, TT, HL, P], FP8)
            # bf16 copy of the first 4 key blocks: window 0's rows average
            # over few keys, so fp8 v error (~6%/sqrt(N_eff)) is too large
            v_ext0 = bigp.tile([P, 4, HL, P], BF16)
            p_sb = bigp.tile([P, TT, HL, WIN], FP8)  # exp(scores), [key, ..., q]
            # zero-init p: DoubleRow AV pairs stream diag-window columns that
            # exp never writes (above-diagonal); they must contribute 0.
            nc.vector.memset(p_sb[:].bitcast(mybir.dt.uint32), 0)
            wv_sb = bigp.tile([P, CK, HC], BF16)
            wp_sb = bigp.tile([P, HC // P, C], BF16)
            wq_sb = bigp.tile([P, NP, CK, P], BF16)
            wk_sb = bigp.tile([P, NP, CK, P], BF16)

            # startup DMAs: the SCALAR ring carries ONLY the tiny biases so
            # the exp stream can start as soon as the first scores land (a
            # weight chunk on this ring would block exp issue for ~30us).
            # sync: x + wk ; gpsimd: wq + wv + wp.
            nc.scalar.dma_start(bq_sb[:], bq_d[:])
            nc.scalar.dma_start(bk_sb[:], bk_d[:])
            nc.scalar.dma_start(bv_sb[:], bv_d[:].rearrange("p (l d) -> p l d", d=D))
            nc.gpsimd.dma_start(bp_sb[:], bp_d[:])
            # x window 0 FIRST on the sync ring (ahead of the wk chunks)
            xT0 = xp.tile([P, CK, WIN], BF16, tag="x", name="x0")
            for ch in range(2):
                nc.sync.dma_start(
                    xT0[:, 4 * ch : 4 * ch + 4], xT_d[0, :, 4 * ch : 4 * ch + 4]
                )
            for j in range(NP):
                for ch in range(2):
                    nc.gpsimd.dma_start(
                        wq_sb[:, j, 4 * ch : 4 * ch + 4], wq_d[:, j, 4 * ch : 4 * ch + 4]
                    )
                    nc.sync.dma_start(
                        wk_sb[:, j, 4 * ch : 4 * ch + 4], wk_d[:, j, 4 * ch : 4 * ch + 4]
                    )
            for ch in range(4):
                nc.gpsimd.dma_start(
                    wv_sb[:, 2 * ch : 2 * ch + 2], wv_d[:, 2 * ch : 2 * ch + 2]
                )
            nc.gpsimd.dma_start(wp_sb[:], wp_d[:])
            # ones columns of v_ext: even slots cols 64:128, odd slots cols 0:64
            nc.gpsimd.memset(
                v_ext[:].rearrange("p t (j c) d -> p t j c d", c=2)[:, :, :, 0, D:],
                1.0,
            )
            nc.gpsimd.memset(
                v_ext[:].rearrange("p t (j c) d -> p t j c d", c=2)[:, :, :, 1, :D],
                1.0,
            )
            nc.gpsimd.memset(
                v_ext0[:].rearrange("p t (j c) d -> p t j c d", c=2)[:, :, :, 0, D:],
                1.0,
            )
            nc.gpsimd.memset(
                v_ext0[:].rearrange("p t (j c) d -> p t j c d", c=2)[:, :, :, 1, :D],
                1.0,
            )

            # DRAM staging for per-window pair ReduceScatter.
            # z chunk row t of window w lives at (p=t%128, idx=(t%512)//128).
            # Last window is split into two half tensors so its two RS ops
            # (collective inputs must be contiguous) can fire separately.
            z_drams = {}
            rs_outs = {}
            for w in range(NW):
                if w == NW - 1:
                    for half in range(2):
                        z_drams[(w, half)] = dramp.tile(
                            [P, 2, C], BF16, name=f"z_dram{w}_{half}"
                        )
                        rs_outs[(w, half)] = dramp.tile(
                            [P // 2, 2, C], BF16, name=f"rs_out{w}_{half}"
                        )
                else:
                    z_drams[w] = dramp.tile([P, WIN // P, C], BF16, name=f"z_dram{w}")
                    rs_outs[w] = dramp.tile(
                        [P // 2, WIN // P, C], BF16, name=f"rs_out{w}"
                    )

            def x_load(s):
                xT = xp.tile([P, CK, WIN], BF16, tag="x", name=f"x{s}")
                for ch in range(2):
                    nc.sync.dma_start(
                        xT[:, 4 * ch : 4 * ch + 4], xT_d[s, :, 4 * ch : 4 * ch + 4]
                    )
                return xT

            def qk_unit(s, xT, qT_w, j):
                """q+k projection chains for head-pair j of t-window s."""
                for w_sb, b_sb, dst in (
                    (wq_sb, bq_sb, qT_w[:, j, :]),
                    (wk_sb, bk_sb, kT[:, j, s * WIN : (s + 1) * WIN]),
                ):
                    ps = mm_ps.tile([P, WIN], F32, tag="mm", name=f"qk{s}{j}")
                    for ck in range(CK):
                        nc.tensor.matmul(
                            ps[:],
                            w_sb[:, j, ck, :],
                            xT[:, ck, :],
                            start=(ck == 0),
                            stop=(ck == CK - 1),
                        )
                    nc.vector.tensor_add(
                        out=dst,
                        in0=ps[:],
                        in1=b_sb[:, j : j + 1].to_broadcast((P, WIN)),
                    )

            def v_unit(s, xT):
                """v projection chains for t-window s -> v_ext key blocks."""
                for tl in range(4):
                    kb = 4 * s + tl
                    ps = mm_ps.tile([P, HC], F32, tag="mm", name=f"v{kb}")
                    for ck in range(CK):
                        nc.tensor.matmul(
                            ps[:],
                            xT[:, ck, tl * P : (tl + 1) * P],
                            wv_sb[:, ck, :],
                            start=(ck == 0),
                            stop=(ck == CK - 1),
                        )
                    # scatter (j,h,d) -> v_ext[:, kb, 2j+h, 64h + d]
                    vdst = v_ext[:, kb].rearrange("p (j c) d -> p j c d", c=2)
                    psv = ps[:].rearrange("p (j c d) -> p j c d", c=2, d=D)
                    bvv = bv_sb[:].rearrange("p (j c) d -> p j c d", c=2)
                    nc.vector.tensor_add(
                        out=vdst[:, :, 0, 0:D], in0=psv[:, :, 0], in1=bvv[:, :, 0]
                    )
                    nc.vector.tensor_add(
                        out=vdst[:, :, 1, D:], in0=psv[:, :, 1], in1=bvv[:, :, 1]
                    )
                    if s == 0:  # bf16 copy for window 0's AV chains
                        v0dst = v_ext0[:, kb].rearrange("p (j c) d -> p j c d", c=2)
                        nc.vector.tensor_add(
                            out=v0dst[:, :, 0, 0:D], in0=psv[:, :, 0], in1=bvv[:, :, 0]
                        )
                        nc.vector.tensor_add(
                            out=v0dst[:, :, 1, D:], in0=psv[:, :, 1], in1=bvv[:, :, 1]
                        )

            def att_kbs(w, jp, qT_w):
                """Scores+exp+mask stream for window w, head-pair pair jp."""
                if True:
                    for kb in range(4 * w + 4):
                        c0 = max(0, kb * P - w * WIN)
                        for jl in range(2):
                            j = 2 * jp + jl
                            sc = score_ps.tile(
                                [P, 2, WIN], F32, tag="sc", name=f"sc{w}{j}{kb}"
                            )
                            for h in range(2):
                                nc.tensor.matmul(
                                    sc[:, h, c0:WIN],
                                    kT[h * D : (h + 1) * D, j, kb * P : (kb + 1) * P],
                                    qT_w[h * D : (h + 1) * D, j, c0:WIN],
                                    start=True,
                                    stop=True,
                                )
                            nc.scalar.activation(
                                out=p_sb[:, kb, 2 * j : 2 * j + 2, c0:WIN],
                                in_=sc[:, :, c0:WIN],
                                func=mybir.ActivationFunctionType.Exp,
                                scale=SCALE,
                            )
                            if kb >= 4 * w:  # diagonal block: zero upper tri
                                if w < 2:
                                    # gpsimd queue is busy with startup weight
                                    # DMA triggers early on -> mask on DVE
                                    nc.vector.tensor_mul(
                                        out=p_sb[:, kb, 2 * j : 2 * j + 2, c0 : c0 + P],
                                        in0=p_sb[:, kb, 2 * j : 2 * j + 2, c0 : c0 + P],
                                        in1=trimask[:, None, :].to_broadcast((P, 2, P)),
                                    )
                                else:
                                    # keep p[k, slot, col] where col >= k
                                    nc.gpsimd.affine_select(
                                        out=p_sb[:, kb, 2 * j : 2 * j + 2, c0 : c0 + P],
                                        in_=p_sb[:, kb, 2 * j : 2 * j + 2, c0 : c0 + P],
                                        compare_op=mybir.AluOpType.is_ge,
                                        fill=0.0,
                                        base=0,
                                        pattern=[[0, 2], [1, P]],
                                        channel_multiplier=-1,
                                    )

            def att_avs(w, jp, yT_w):
                """AV chains + softmax-denominator normalize for jp."""
                if True:
                    for jl in range(2):
                        j = 2 * jp + jl
                        # Newton reciprocal scratch: slot 0 = r0 (magic seed),
                        # slot 1 = u = den*r0. negr1 = (u-2)*r0 = -1/den.
                        nw = nwp.tile([P, 2, 2, WIN], F32, tag="nw", name=f"nw{w}{j}")
                        r_raw = rp.tile([P, 2, WIN], BF16, tag="rb", name=f"rb{w}{j}")
                        r_sh = rp.tile([P, 2, WIN], BF16, tag="rs", name=f"rs{w}{j}")
                        for h in range(2):
                            av = av_ps.tile([P, WIN], F32, tag="av", name=f"av{w}{j}{h}")
                            if w == 0:
                                for kb in range(4):
                                    c0 = kb * P
                                    nc.tensor.matmul(
                                        av[:, c0:WIN],
                                        v_ext0[:, kb, 2 * j + h, :],
                                        p_sb[:, kb, 2 * j + h, c0:WIN],
                                        start=(kb == 0),
                                        stop=(kb == 3),
                                    )
                            else:
                                npair = 2 * w + 2
                                for pr in range(npair):
                                    kb = 2 * pr
                                    c0 = max(0, kb * P - w * WIN)
                                    nc.tensor.matmul(
                                        av[:, c0:WIN],
                                        v_ext[:, kb : kb + 2, 2 * j + h, :],
                                        p_sb[:, kb : kb + 2, 2 * j + h, c0:WIN],
                                        start=(pr == 0),
                                        stop=(pr == npair - 1),
                                        perf_mode=mybir.MatmulPerfMode.DoubleRow,
                                    )
                            # den replicas: h=0 rows 64:128, h=1 rows 0:64
                            dl, dh = (D, P) if h == 0 else (0, D)
                            al, ah = (0, D) if h == 0 else (D, P)
                            # r0_bits = MAGIC - den_bits  (magic recip seed)
                            nc.vector.tensor_scalar(
                                out=nw[dl:dh, h, 0, :].bitcast(mybir.dt.int32),
                                in0=av[dl:dh, :].bitcast(mybir.dt.int32),
                                scalar1=-1,
                                scalar2=0x7EF311C3,
                                op0=mybir.AluOpType.mult,
                                op1=mybir.AluOpType.add,
                            )
                            nc.vector.tensor_mul(
                                out=nw[dl:dh, h, 1, :],
                                in0=av[dl:dh, :],
                                in1=nw[dl:dh, h, 0, :],
                            )
                            with nc.allow_low_precision(reason="recip in bf16"):
                                nc.vector.scalar_tensor_tensor(
                                    out=r_raw[dl:dh, h, :],
                                    in0=nw[dl:dh, h, 1, :],
                                    scalar=2.0,
                                    in1=nw[dl:dh, h, 0, :],
                                    op0=mybir.AluOpType.subtract,
                                    op1=mybir.AluOpType.mult,
                                )
                            nc.sync.dma_start(r_sh[al:ah, h, :], r_raw[dl:dh, h, :])
                            with nc.allow_low_precision(reason="y in bf16"):
                                nc.vector.scalar_tensor_tensor(
                                    out=yT_w[al:ah, j, :],
                                    in0=av[al:ah, :],
                                    scalar=-1.0,
                                    in1=r_sh[al:ah, h, :],
                                    op0=mybir.AluOpType.mult,
                                    op1=mybir.AluOpType.mult,
                                )

            def proj(w, yT_w, half=None):
                if half is None or half == 0:
                    z_sb = zoutp.tile([P, WIN // P, C], BF16, tag="z", name=f"z{w}")
                    proj._z = z_sb
                else:
                    z_sb = proj._z
                tls = range(WIN // P) if half is None else range(2 * half, 2 * half + 2)
                for tl in tls:
                    for n in range(2):
                        ps = mm_ps.tile([P, 512], F32, tag="mm", name=f"pj{w}{tl}{n}")
                        for cb in range(HC // P):
                            nc.tensor.matmul(
                                ps[:],
                                yT_w[:, cb, tl * P : (tl + 1) * P],
                                wp_sb[:, cb, n * 512 : (n + 1) * 512],
                                start=(cb == 0),
                                stop=(cb == HC // P - 1),
                            )
                        nc.vector.tensor_add(
                            out=z_sb[:, tl, n * 512 : (n + 1) * 512],
                            in0=ps[:],
                            in1=bp_sb[:, n * 512 : (n + 1) * 512],
                        )
                    if tl % 2:
                        if w == NW - 1:
                            nc.sync.dma_start(
                                z_drams[(w, tl // 2)][:], z_sb[:, tl - 1 : tl + 1, :]
                            )
                        else:
                            nc.sync.dma_start(
                                z_drams[w][:, tl - 1 : tl + 1, :],
                                z_sb[:, tl - 1 : tl + 1, :],
                            )

            def rs_cc(key):
                nc.gpsimd.collective_compute(
                    "ReduceScatter",
                    mybir.AluOpType.add,
                    replica_groups=[[0, 1], [2, 3], [4, 5], [6, 7]],
                    ins=[z_drams[key][:].opt()],
                    outs=[rs_outs[key][:].opt()],
                )

            def rs_out_dma(key):
                w, half = key if isinstance(key, tuple) else (key, None)
                k0 = 0 if half is None else 2 * half
                k1 = WIN // P if half is None else 2 * half + 2
                o0 = w * (WIN // 2)
                dst = out_d[o0 : o0 + WIN // 2, :].rearrange(
                    "(p k) c -> p k c", k=WIN // P
                )
                nc.gpsimd.dma_start(dst[:, k0:k1, :], rs_outs[key][:])

            # Hand-pipelined emission: the Tile scheduler follows emission
            # priority among ready instructions, so units are emitted in the
            # order they should execute. PE filler (next qkv pieces, v
            # chains) is emitted right after each jp's score/exp stream so
            # it runs in the PE idle slots of the NEXT exp-bound phase.
            # Collective trigger one window late, its out-DMA two windows
            # late (so neither input-wait blocks gpsimd-queue work).
            qT_tiles, x_tiles, yT_tiles = {}, {}, {}
            x_tiles[0] = xT0
            qT_tiles[0] = qtp.tile([P, NP, WIN], BF16, tag="qt", name="qt0")
            qk_unit(0, x_tiles[0], qT_tiles[0], 0)
            qk_unit(0, x_tiles[0], qT_tiles[0], 1)
            for w in range(NW):
                yT_tiles[w] = ytp.tile([P, NP, WIN], BF16, tag="yt", name=f"yt{w}")
                att_kbs(w, 0, qT_tiles[w])
                if w == 0:
                    qk_unit(0, x_tiles[0], qT_tiles[0], 2)
                    qk_unit(0, x_tiles[0], qT_tiles[0], 3)
                    v_unit(0, x_tiles[0])
                att_avs(w, 0, yT_tiles[w])
                att_kbs(w, 1, qT_tiles[w])
                if w + 1 < NW:
                    x_tiles[w + 1] = x_load(w + 1)
                    qT_tiles[w + 1] = qtp.tile(
                        [P, NP, WIN], BF16, tag="qt", name=f"qt{w + 1}"
                    )
                    qk_unit(w + 1, x_tiles[w + 1], qT_tiles[w + 1], 0)
                    qk_unit(w + 1, x_tiles[w + 1], qT_tiles[w + 1], 1)
                if w >= 1:
                    rs_cc(w - 1)
                if w >= 2:
                    rs_out_dma(w - 2)
                att_avs(w, 1, yT_tiles[w])
                if w + 1 < NW:
                    qk_unit(w + 1, x_tiles[w + 1], qT_tiles[w + 1], 2)
                    qk_unit(w + 1, x_tiles[w + 1], qT_tiles[w + 1], 3)
                    v_unit(w + 1, x_tiles[w + 1])
                if w == NW - 1:
                    # tail: fire each half's RS as soon as its z lands
                    proj(w, yT_tiles[w], half=0)
                    rs_cc((w, 0))
                    proj(w, yT_tiles[w], half=1)
                    rs_cc((w, 1))
                    rs_out_dma(w - 1)
                    rs_out_dma((w, 0))
                    rs_out_dma((w, 1))
                else:
                    proj(w, yT_tiles[w])

    nc.compile()
    return nc


def _in_maps(inputs):
    x = np.asarray(inputs["x"], dtype=np.float32)
    w_attn = np.asarray(inputs["w_attn"], dtype=np.float32)
    b_attn = np.asarray(inputs["b_attn"], dtype=np.float32)
    w_proj = np.asarray(inputs["w_proj"], dtype=np.float32)
    b_proj = np.asarray(inputs["b_proj"], dtype=np.float32)

    maps = []
    for core in range(N_CORES):
        b, g = core // 2, core % 2
        s = g * HC
        # x [T, C] -> x^T [ci, ck, t] with c = ck*128+ci
        xT = (
            x[b]
            .T.reshape(CK, P, NW, WIN)
            .transpose(2, 1, 0, 3)
            .astype(ml_dtypes.bfloat16)
        )
        # [C, HC] -> [ki, j, ko, n] with c = ko*128+ki, qcol = j*128+n
        wq = (
            w_attn[:, s : s + HC]
            .reshape(CK, P, NP, P)
            .transpose(1, 2, 0, 3)
            .astype(ml_dtypes.bfloat16)
        )
        wk = (
            w_attn[:, C + s : C + s + HC]
            .reshape(CK, P, NP, P)
            .transpose(1, 2, 0, 3)
            .astype(ml_dtypes.bfloat16)
        )
        # [C, HC] -> [ki, ko, vcol]
        wv = (
            w_attn[:, 2 * C + s : 2 * C + s + HC]
            .reshape(CK, P, HC)
            .transpose(1, 0, 2)
            .astype(ml_dtypes.bfloat16)
        )
        # [HC, C] -> [ki, ko, co]
        wp = (
            w_proj[s : s + HC, :]
            .reshape(HC // P, P, C)
            .transpose(1, 0, 2)
            .astype(ml_dtypes.bfloat16)
        )
        bq = b_attn[s : s + HC].reshape(NP, P).T
        bk = b_attn[C + s : C + s + HC].reshape(NP, P).T
        bv = np.broadcast_to(
            b_attn[2 * C + s : 2 * C + s + HC].astype(ml_dtypes.bfloat16), (P, HC)
        )
        bp = (
            np.broadcast_to(b_proj.astype(ml_dtypes.bfloat16), (P, C))
            if g == 0
            else np.zeros((P, C), ml_dtypes.bfloat16)
        )
        maps.append(
            {
                "xT": np.ascontiguousarray(xT),
                "wq": np.ascontiguousarray(wq),
                "wk": np.ascontiguousarray(wk),
                "wv": np.ascontiguousarray(wv),
                "wp": np.ascontiguousarray(wp),
                "bq": np.ascontiguousarray(bq),
                "bk": np.ascontiguousarray(bk),
                "bv": np.ascontiguousarray(bv),
                "bp": np.ascontiguousarray(bp),
            }
        )
    return maps


def _run(inputs, trace=False, trace_cores=None):
    if "nc" not in _CACHE:
        _CACHE["nc"] = _build_nc()
    nc = _CACHE["nc"]
    res = run_bass_kernel_spmd(
        nc,
        _in_maps(inputs),
        list(range(N_CORES)),
        trace=trace,
        trace_cores=trace_cores,
    )
    # window w rows [512w, 512w+512): row t at (p=t%128, k=(t%512)//128);
    # even core holds p<64, odd core p>=64; lands at out[256w + 64k + ...]
    out = np.empty((B, T, C), np.float32)
    for b in range(B):
        ev = res.results[2 * b]["out"].astype(np.float32)
        od = res.results[2 * b + 1]["out"].astype(np.float32)
        for w in range(NW):
            o0, k = w * (WIN // 2), WIN // P
            for g, core_out in ((0, ev), (1, od)):
                blk = core_out[o0 : o0 + WIN // 2].reshape(64, k, C)
                dst = out[b, w * WIN : (w + 1) * WIN].reshape(k, 128, C)
                dst[:, 64 * g : 64 * g + 64, :] = blk.transpose(1, 0, 2)
    return out, res


def kernel(**inputs):
    out, _ = _run(inputs)
    return out


# revision 57
# speedup vs baseline: 1.0472x; 1.0472x over previous
"""Causal self-attention (B=4, T=2048, C=1024, H=16) on 8 Trainium2 cores.

Sharding: core c -> batch b = c//2, head-group g = c%2 (8 local heads,
tensor-parallel). Partial c_proj outputs of a pair are combined with one
ReduceScatter per 512-row window; host reassembles.

Schedule is window-major (4 windows of 512 query rows) so the Scalar
engine's exp stream - the hard resource floor (~16.8M elements/core at
1 elem/cycle/lane) - starts as soon as the first QKV window is done and
runs continuously. Per window and head-pair-pair (j-pair), score matmuls
for 4 head-slots land in one 4-bank PSUM tile and a single N=2048 exp
evacuates them (minimizes ACT per-instruction overhead).

AV matmuls run "flipped": stationary = [v | ones] (M=128, FWL-friendly),
moving = p (N=512 bf16->fp8 stream). Output rows 0-63 are y^T for the
head, rows 64-127 are the softmax denominator replicated 64x (the ones
columns), so no PE transpose and no cross-partition reduction is needed.
The denominator reciprocal is computed by DVE, shifted to the opposite
partition half by a small SBUF->SBUF DMA, and fused into the PSUM->SBUF
evacuation multiply that writes y^T.

p is stored as fp8e4 (e4m3) to fit SBUF; scores/weights stay bf16.

Self-contained: only imports concourse (installed library) + numpy.
"""

import ml_dtypes
import numpy as np

import concourse.bass_utils as _bass_utils
import concourse.mybir as mybir
import concourse.tile as tile
from concourse import bacc
from concourse.bass_utils import run_bass_kernel_spmd

del _bass_utils

B, T, C = 4, 2048, 1024
H_TOTAL, D = 16, 64
N_CORES = 8
HL = H_TOTAL // 2  # local heads per core (8)
HC = HL * D  # local head cols (512)
NP = HL // 2  # head pairs (4)
P = 128
CK = C // P  # 8 contraction chunks for qkv
WIN = 512
NW = T // WIN  # 4 query windows of 512
TT = T // P  # 16 key blocks of 128
F32 = mybir.dt.float32
BF16 = mybir.dt.bfloat16
FP8 = mybir.dt.float8e4
SCALE = 1.0 / 8.0  # 1/sqrt(D)

_CACHE = {}


def _build_nc():
    nc = bacc.Bacc("TRN2", target_bir_lowering=False, debug=False, num_devices=N_CORES)

    xT_d = nc.dram_tensor("xT", [NW, P, CK, WIN], BF16, kind="ExternalInput")
    wq_d = nc.dram_tensor("wq", [P, NP, CK, P], BF16, kind="ExternalInput")
    wk_d = nc.dram_tensor("wk", [P, NP, CK, P], BF16, kind="ExternalInput")
    wv_d = nc.dram_tensor("wv", [P, CK, HC], BF16, kind="ExternalInput")
    bq_d = nc.dram_tensor("bq", [P, NP], F32, kind="ExternalInput")
    bk_d = nc.dram_tensor("bk", [P, NP], F32, kind="ExternalInput")
    bv_d = nc.dram_tensor("bv", [P, HC], BF16, kind="ExternalInput")
    wp_d = nc.dram_tensor("wp", [P, HC // P, C], BF16, kind="ExternalInput")
    bp_d = nc.dram_tensor("bp", [P, C], BF16, kind="ExternalInput")
    out_d = nc.dram_tensor("out", [T // 2, C], BF16, kind="ExternalOutput")

    with tile.TileContext(nc) as tc:
        with (
            tc.tile_pool(name="const", bufs=1) as constp,
            tc.tile_pool(name="big", bufs=1) as bigp,
            tc.tile_pool(name="xp", bufs=2) as xp,
            tc.tile_pool(name="qtp", bufs=2) as qtp,
            tc.tile_pool(name="ytp", bufs=2) as ytp,
            tc.tile_pool(name="rp", bufs=2) as rp,
            tc.tile_pool(name="nwp", bufs=1) as nwp,
            tc.tile_pool(name="zout", bufs=1) as zoutp,
            tc.tile_pool(name="score_ps", bufs=2, space="PSUM") as score_ps,
            tc.tile_pool(name="av_ps", bufs=2, space="PSUM") as av_ps,
            tc.tile_pool(name="mm_ps", bufs=2, space="PSUM") as mm_ps,
            tc.tile_pool(name="dram", bufs=1, space="DRAM") as dramp,
        ):
            # ---- constants ----
            # trimask[k, q] = 1 where q >= k else 0 (built first: the gpsimd
            # queue is about to be loaded with weight-chunk DMA triggers)
            trif = constp.tile([P, P], F32)
            nc.gpsimd.memset(trif, 1.0)
            nc.gpsimd.affine_select(
                out=trif,
                in_=trif,
                compare_op=mybir.AluOpType.is_ge,
                fill=0.0,
                base=0,
                pattern=[[1, P]],
                channel_multiplier=-1,
            )
            trimask = constp.tile([P, P], BF16)
            nc.vector.tensor_copy(out=trimask[:], in_=trif[:])
            bq_sb = constp.tile([P, NP], F32)
            bk_sb = constp.tile([P, NP], F32)
            bv_sb = constp.tile([P, HL, D], BF16)
            bp_sb = constp.tile([P, C], BF16)

            # ---- persistent tensors ----
            kT = bigp.tile([P, NP, T], BF16)  # k^T [kcol, t] (all key windows)
            # v_ext[k, kb, 2j+h, :]: h=0 -> [v(0:64) | ones], h=1 -> [ones | v(64:128)]
            # fp8 so AV matmuls can run DoubleRow (2 key blocks per matmul)
            v_ext = bigp.tile([P, TT, HL, P], FP8)
            # bf16 copy of the first 4 key blocks: window 0's rows average
            # over few keys, so fp8 v error (~6%/sqrt(N_eff)) is too large
            v_ext0 = bigp.tile([P, 4, HL, P], BF16)
            p_sb = bigp.tile([P, TT, HL, WIN], FP8)  # exp(scores), [key, ..., q]
            # zero-init p: DoubleRow AV pairs stream diag-window columns that
            # exp never writes (above-diagonal); they must contribute 0.
            nc.vector.memset(p_sb[:].bitcast(mybir.dt.uint32), 0)
            wv_sb = bigp.tile([P, CK, HC], BF16)
            wp_sb = bigp.tile([P, HC // P, C], BF16)
            wq_sb = bigp.tile([P, NP, CK, P], BF16)
            wk_sb = bigp.tile([P, NP, CK, P], BF16)

            # startup DMAs: the SCALAR ring carries ONLY the tiny biases so
            # the exp stream can start as soon as the first scores land (a
            # weight chunk on this ring would block exp issue for ~30us).
            # sync: x + wk ; gpsimd: wq + wv + wp.
            nc.scalar.dma_start(bq_sb[:], bq_d[:])
            nc.scalar.dma_start(bk_sb[:], bk_d[:])
            nc.scalar.dma_start(bv_sb[:], bv_d[:].rearrange("p (l d) -> p l d", d=D))
            nc.gpsimd.dma_start(bp_sb[:], bp_d[:])
            # x window 0 FIRST on the sync ring (ahead of the wk chunks)
            xT0 = xp.tile([P, CK, WIN], BF16, tag="x", name="x0")
            for ch in range(2):
                nc.sync.dma_start(
                    xT0[:, 4 * ch : 4 * ch + 4], xT_d[0, :, 4 * ch : 4 * ch + 4]
                )
            for j in range(NP):
                for ch in range(2):
                    nc.gpsimd.dma_start(
                        wq_sb[:, j, 4 * ch : 4 * ch + 4], wq_d[:, j, 4 * ch : 4 * ch + 4]
                    )
                    nc.sync.dma_start(
                        wk_sb[:, j, 4 * ch : 4 * ch + 4], wk_d[:, j, 4 * ch : 4 * ch + 4]
                    )
            for ch in range(4):
                nc.gpsimd.dma_start(
                    wv_sb[:, 2 * ch : 2 * ch + 2], wv_d[:, 2 * ch : 2 * ch + 2]
                )
            nc.gpsimd.dma_start(wp_sb[:], wp_d[:])
            # ones columns of v_ext: even slots cols 64:128, odd slots cols 0:64
            nc.gpsimd.memset(
                v_ext[:].rearrange("p t (j c) d -> p t j c d", c=2)[:, :, :, 0, D:],
                1.0,
            )
            nc.gpsimd.memset(
                v_ext[:].rearrange("p t (j c) d -> p t j c d", c=2)[:, :, :, 1, :D],
                1.0,
            )
            nc.gpsimd.memset(
                v_ext0[:].rearrange("p t (j c) d -> p t j c d", c=2)[:, :, :, 0, D:],
                1.0,
            )
            nc.gpsimd.memset(
                v_ext0[:].rearrange("p t (j c) d -> p t j c d", c=2)[:, :, :, 1, :D],
                1.0,
            )

            # DRAM staging for per-window pair ReduceScatter.
            # z chunk row t of window w lives at (p=t%128, idx=(t%512)//128).
            # Last window is split into two half tensors so its two RS ops
            # (collective inputs must be contiguous) can fire separately.
            z_drams = {}
            rs_outs = {}
            for w in range(NW):
                if w == NW - 1:
                    for half in range(2):
                        z_drams[(w, half)] = dramp.tile(
                            [P, 2, C], BF16, name=f"z_dram{w}_{half}"
                        )
                        rs_outs[(w, half)] = dramp.tile(
                            [P // 2, 2, C], BF16, name=f"rs_out{w}_{half}"
                        )
                else:
                    z_drams[w] = dramp.tile([P, WIN // P, C], BF16, name=f"z_dram{w}")
                    rs_outs[w] = dramp.tile(
                        [P // 2, WIN // P, C], BF16, name=f"rs_out{w}"
                    )

            def x_load(s):
                xT = xp.tile([P, CK, WIN], BF16, tag="x", name=f"x{s}")
                for ch in range(2):
                    nc.sync.dma_start(
                        xT[:, 4 * ch : 4 * ch + 4], xT_d[s, :, 4 * ch : 4 * ch + 4]
                    )
                return xT

            def qk_unit(s, xT, qT_w, j):
                """q+k projection chains for head-pair j of t-window s."""
                for w_sb, b_sb, dst in (
                    (wq_sb, bq_sb, qT_w[:, j, :]),
                    (wk_sb, bk_sb, kT[:, j, s * WIN : (s + 1) * WIN]),
                ):
                    ps = mm_ps.tile([P, WIN], F32, tag="mm", name=f"qk{s}{j}")
                    for ck in range(CK):
                        nc.tensor.matmul(
                            ps[:],
                            w_sb[:, j, ck, :],
                            xT[:, ck, :],
                            start=(ck == 0),
                            stop=(ck == CK - 1),
                        )
                    nc.vector.tensor_add(
                        out=dst,
                        in0=ps[:],
                        in1=b_sb[:, j : j + 1].to_broadcast((P, WIN)),
                    )

            def v_unit(s, xT):
                """v projection chains for t-window s -> v_ext key blocks."""
                for tl in range(4):
                    kb = 4 * s + tl
                    ps = mm_ps.tile([P, HC], F32, tag="mm", name=f"v{kb}")
                    for ck in range(CK):
                        nc.tensor.matmul(
                            ps[:],
                            xT[:, ck, tl * P : (tl + 1) * P],
                            wv_sb[:, ck, :],
                            start=(ck == 0),
                            stop=(ck == CK - 1),
                        )
                    # scatter (j,h,d) -> v_ext[:, kb, 2j+h, 64h + d]
                    vdst = v_ext[:, kb].rearrange("p (j c) d -> p j c d", c=2)
                    psv = ps[:].rearrange("p (j c d) -> p j c d", c=2, d=D)
                    bvv = bv_sb[:].rearrange("p (j c) d -> p j c d", c=2)
                    nc.vector.tensor_add(
                        out=vdst[:, :, 0, 0:D], in0=psv[:, :, 0], in1=bvv[:, :, 0]
                    )
                    nc.vector.tensor_add(
                        out=vdst[:, :, 1, D:], in0=psv[:, :, 1], in1=bvv[:, :, 1]
                    )
                    if s == 0:  # bf16 copy for window 0's AV chains
                        v0dst = v_ext0[:, kb].rearrange("p (j c) d -> p j c d", c=2)
                        nc.vector.tensor_add(
                            out=v0dst[:, :, 0, 0:D], in0=psv[:, :, 0], in1=bvv[:, :, 0]
                        )
                        nc.vector.tensor_add(
                            out=v0dst[:, :, 1, D:], in0=psv[:, :, 1], in1=bvv[:, :, 1]
                        )

            def att_kbs(w, jp, qT_w):
                """Scores+exp+mask stream for window w, head-pair pair jp."""
                for jl in range(2):
                    j = 2 * jp + jl
                    for kb in range(4 * w + 4):
                        c0 = max(0, kb * P - w * WIN)
                        sc = score_ps.tile(
                            [P, 2, WIN], F32, tag="sc", name=f"sc{w}{j}{kb}"
                        )
                        for h in range(2):
                            nc.tensor.matmul(
                                sc[:, h, c0:WIN],
                                kT[h * D : (h + 1) * D, j, kb * P : (kb + 1) * P],
                                qT_w[h * D : (h + 1) * D, j, c0:WIN],
                                start=True,
                                stop=True,
                            )
                        nc.scalar.activation(
                            out=p_sb[:, kb, 2 * j : 2 * j + 2, c0:WIN],
                            in_=sc[:, :, c0:WIN],
                            func=mybir.ActivationFunctionType.Exp,
                            scale=SCALE,
                        )
                        if kb >= 4 * w:  # diagonal block: zero upper tri
                            if w < 2:
                                # gpsimd queue is busy with startup weight
                                # DMA triggers early on -> mask on DVE
                                nc.vector.tensor_mul(
                                    out=p_sb[:, kb, 2 * j : 2 * j + 2, c0 : c0 + P],
                                    in0=p_sb[:, kb, 2 * j : 2 * j + 2, c0 : c0 + P],
                                    in1=trimask[:, None, :].to_broadcast((P, 2, P)),
                                )
                            else:
                                # keep p[k, slot, col] where col >= k
                                nc.gpsimd.affine_select(
                                    out=p_sb[:, kb, 2 * j : 2 * j + 2, c0 : c0 + P],
                                    in_=p_sb[:, kb, 2 * j : 2 * j + 2, c0 : c0 + P],
                                    compare_op=mybir.AluOpType.is_ge,
                                    fill=0.0,
                                    base=0,
                                    pattern=[[0, 2], [1, P]],
                                    channel_multiplier=-1,
                                )

            def att_avs(w, jp, yT_w):
                """AV chains + softmax-denominator normalize for jp."""
                if True:
                    for jl in range(2):
                        j = 2 * jp + jl
                        # Newton reciprocal scratch: slot 0 = r0 (magic seed),
                        # slot 1 = u = den*r0. negr1 = (u-2)*r0 = -1/den.
                        nw = nwp.tile([P, 2, 2, WIN], F32, tag="nw", name=f"nw{w}{j}")
                        r_raw = rp.tile([P, 2, WIN], BF16, tag="rb", name=f"rb{w}{j}")
                        r_sh = rp.tile([P, 2, WIN], BF16, tag="rs", name=f"rs{w}{j}")
                        for h in range(2):
                            av = av_ps.tile([P, WIN], F32, tag="av", name=f"av{w}{j}{h}")
                            if w == 0:
                                for kb in range(4):
                                    c0 = kb * P
                                    nc.tensor.matmul(
                                        av[:, c0:WIN],
                                        v_ext0[:, kb, 2 * j + h, :],
                                        p_sb[:, kb, 2 * j + h, c0:WIN],
                                        start=(kb == 0),
                                        stop=(kb == 3),
                                    )
                            else:
                                npair = 2 * w + 2
                                for pr in range(npair):
                                    kb = 2 * pr
                                    c0 = max(0, kb * P - w * WIN)
                                    nc.tensor.matmul(
                                        av[:, c0:WIN],
                                        v_ext[:, kb : kb + 2, 2 * j + h, :],
                                        p_sb[:, kb : kb + 2, 2 * j + h, c0:WIN],
                                        start=(pr == 0),
                                        stop=(pr == npair - 1),
                                        perf_mode=mybir.MatmulPerfMode.DoubleRow,
                                    )
                            # den replicas: h=0 rows 64:128, h=1 rows 0:64
                            dl, dh = (D, P) if h == 0 else (0, D)
                            al, ah = (0, D) if h == 0 else (D, P)
                            # r0_bits = MAGIC - den_bits  (magic recip seed)
                            nc.vector.tensor_scalar(
                                out=nw[dl:dh, h, 0, :].bitcast(mybir.dt.int32),
                                in0=av[dl:dh, :].bitcast(mybir.dt.int32),
                                scalar1=-1,
                                scalar2=0x7EF311C3,
                                op0=mybir.AluOpType.mult,
                                op1=mybir.AluOpType.add,
                            )
                            nc.vector.tensor_mul(
                                out=nw[dl:dh, h, 1, :],
                                in0=av[dl:dh, :],
                                in1=nw[dl:dh, h, 0, :],
                            )
                            with nc.allow_low_precision(reason="recip in bf16"):
                                nc.vector.scalar_tensor_tensor(
                                    out=r_raw[dl:dh, h, :],
                                    in0=nw[dl:dh, h, 1, :],
                                    scalar=2.0,
                                    in1=nw[dl:dh, h, 0, :],
                                    op0=mybir.AluOpType.subtract,
                                    op1=mybir.AluOpType.mult,
                                )
                            nc.sync.dma_start(r_sh[al:ah, h, :], r_raw[dl:dh, h, :])
                            with nc.allow_low_precision(reason="y in bf16"):
                                nc.vector.scalar_tensor_tensor(
                                    out=yT_w[al:ah, j, :],
                                    in0=av[al:ah, :],
                                    scalar=-1.0,
                                    in1=r_sh[al:ah, h, :],
                                    op0=mybir.AluOpType.mult,
                                    op1=mybir.AluOpType.mult,
                                )

            def proj(w, yT_w, half=None):
                if half is None or half == 0:
                    z_sb = zoutp.tile([P, WIN // P, C], BF16, tag="z", name=f"z{w}")
                    proj._z = z_sb
                else:
                    z_sb = proj._z
                tls = range(WIN // P) if half is None else range(2 * half, 2 * half + 2)
                for tl in tls:
                    for n in range(2):
                        ps = mm_ps.tile([P, 512], F32, tag="mm", name=f"pj{w}{tl}{n}")
                        for cb in range(HC // P):
                            nc.tensor.matmul(
                                ps[:],
                                yT_w[:, cb, tl * P : (tl + 1) * P],
                                wp_sb[:, cb, n * 512 : (n + 1) * 512],
                                start=(cb == 0),
                                stop=(cb == HC // P - 1),
                            )
                        nc.vector.tensor_add(
                            out=z_sb[:, tl, n * 512 : (n + 1) * 512],
                            in0=ps[:],
                            in1=bp_sb[:, n * 512 : (n + 1) * 512],
                        )
                    if tl % 2:
                        if w == NW - 1:
                            nc.sync.dma_start(
                                z_drams[(w, tl // 2)][:], z_sb[:, tl - 1 : tl + 1, :]
                            )
                        else:
                            nc.sync.dma_start(
                                z_drams[w][:, tl - 1 : tl + 1, :],
                                z_sb[:, tl - 1 : tl + 1, :],
                            )

            def rs_cc(key):
                nc.gpsimd.collective_compute(
                    "ReduceScatter",
                    mybir.AluOpType.add,
                    replica_groups=[[0, 1], [2, 3], [4, 5], [6, 7]],
                    ins=[z_drams[key][:].opt()],
                    outs=[rs_outs[key][:].opt()],
                )

            def rs_out_dma(key):
                w, half = key if isinstance(key, tuple) else (key, None)
                k0 = 0 if half is None else 2 * half
                k1 = WIN // P if half is None else 2 * half + 2
                o0 = w * (WIN // 2)
                dst = out_d[o0 : o0 + WIN // 2, :].rearrange(
                    "(p k) c -> p k c", k=WIN // P
                )
                nc.gpsimd.dma_start(dst[:, k0:k1, :], rs_outs[key][:])

            # Hand-pipelined emission: the Tile scheduler follows emission
            # priority among ready instructions, so units are emitted in the
            # order they should execute. PE filler (next qkv pieces, v
            # chains) is emitted right after each jp's score/exp stream so
            # it runs in the PE idle slots of the NEXT exp-bound phase.
            # Collective trigger one window late, its out-DMA two windows
            # late (so neither input-wait blocks gpsimd-queue work).
            qT_tiles, x_tiles, yT_tiles = {}, {}, {}
            x_tiles[0] = xT0
            qT_tiles[0] = qtp.tile([P, NP, WIN], BF16, tag="qt", name="qt0")
            qk_unit(0, x_tiles[0], qT_tiles[0], 0)
            qk_unit(0, x_tiles[0], qT_tiles[0], 1)
            for w in range(NW):
                yT_tiles[w] = ytp.tile([P, NP, WIN], BF16, tag="yt", name=f"yt{w}")
                att_kbs(w, 0, qT_tiles[w])
                if w == 0:
                    qk_unit(0, x_tiles[0], qT_tiles[0], 2)
                    qk_unit(0, x_tiles[0], qT_tiles[0], 3)
                    v_unit(0, x_tiles[0])
                att_avs(w, 0, yT_tiles[w])
                att_kbs(w, 1, qT_tiles[w])
                if w + 1 < NW:
                    x_tiles[w + 1] = x_load(w + 1)
                    qT_tiles[w + 1] = qtp.tile(
                        [P, NP, WIN], BF16, tag="qt", name=f"qt{w + 1}"
                    )
                    qk_unit(w + 1, x_tiles[w + 1], qT_tiles[w + 1], 0)
                    qk_unit(w + 1, x_tiles[w + 1], qT_tiles[w + 1], 1)
                if w >= 1:
                    rs_cc(w - 1)
                if w >= 2:
                    rs_out_dma(w - 2)
                att_avs(w, 1, yT_tiles[w])
                if w + 1 < NW:
                    qk_unit(w + 1, x_tiles[w + 1], qT_tiles[w + 1], 2)
                    qk_unit(w + 1, x_tiles[w + 1], qT_tiles[w + 1], 3)
                    v_unit(w + 1, x_tiles[w + 1])
                if w == NW - 1:
                    # tail: fire each half's RS as soon as its z lands
                    proj(w, yT_tiles[w], half=0)
                    rs_cc((w, 0))
                    proj(w, yT_tiles[w], half=1)
                    rs_cc((w, 1))
                    rs_out_dma(w - 1)
                    rs_out_dma((w, 0))
                    rs_out_dma((w, 1))
                else:
                    proj(w, yT_tiles[w])

    nc.compile()
    return nc


def _in_maps(inputs):
    x = np.asarray(inputs["x"], dtype=np.float32)
    w_attn = np.asarray(inputs["w_attn"], dtype=np.float32)
    b_attn = np.asarray(inputs["b_attn"], dtype=np.float32)
    w_proj = np.asarray(inputs["w_proj"], dtype=np.float32)
    b_proj = np.asarray(inputs["b_proj"], dtype=np.float32)

    maps = []
    for core in range(N_CORES):
        b, g = core // 2, core % 2
        s = g * HC
        # x [T, C] -> x^T [ci, ck, t] with c = ck*128+ci
        xT = (
            x[b]
            .T.reshape(CK, P, NW, WIN)
            .transpose(2, 1, 0, 3)
            .astype(ml_dtypes.bfloat16)
        )
        # [C, HC] -> [ki, j, ko, n] with c = ko*128+ki, qcol = j*128+n
        wq = (
            w_attn[:, s : s + HC]
            .reshape(CK, P, NP, P)
            .transpose(1, 2, 0, 3)
            .astype(ml_dtypes.bfloat16)
        )
        wk = (
            w_attn[:, C + s : C + s + HC]
            .reshape(CK, P, NP, P)
            .transpose(1, 2, 0, 3)
            .astype(ml_dtypes.bfloat16)
        )
        # [C, HC] -> [ki, ko, vcol]
        wv = (
            w_attn[:, 2 * C + s : 2 * C + s + HC]
            .reshape(CK, P, HC)
            .transpose(1, 0, 2)
            .astype(ml_dtypes.bfloat16)
        )
        # [HC, C] -> [ki, ko, co]
        wp = (
            w_proj[s : s + HC, :]
            .reshape(HC // P, P, C)
            .transpose(1, 0, 2)
            .astype(ml_dtypes.bfloat16)
        )
        bq = b_attn[s : s + HC].reshape(NP, P).T
        bk = b_attn[C + s : C + s + HC].reshape(NP, P).T
        bv = np.broadcast_to(
            b_attn[2 * C + s : 2 * C + s + HC].astype(ml_dtypes.bfloat16), (P, HC)
        )
        bp = (
            np.broadcast_to(b_proj.astype(ml_dtypes.bfloat16), (P, C))
            if g == 0
            else np.zeros((P, C), ml_dtypes.bfloat16)
        )
        maps.append(
            {
                "xT": np.ascontiguousarray(xT),
                "wq": np.ascontiguousarray(wq),
                "wk": np.ascontiguousarray(wk),
                "wv": np.ascontiguousarray(wv),
                "wp": np.ascontiguousarray(wp),
                "bq": np.ascontiguousarray(bq),
                "bk": np.ascontiguousarray(bk),
                "bv": np.ascontiguousarray(bv),
                "bp": np.ascontiguousarray(bp),
            }
        )
    return maps


def _run(inputs, trace=False, trace_cores=None):
    if "nc" not in _CACHE:
        _CACHE["nc"] = _build_nc()
    nc = _CACHE["nc"]
    res = run_bass_kernel_spmd(
        nc,
        _in_maps(inputs),
        list(range(N_CORES)),
        trace=trace,
        trace_cores=trace_cores,
    )
    # window w rows [512w, 512w+512): row t at (p=t%128, k=(t%512)//128);
    # even core holds p<64, odd core p>=64; lands at out[256w + 64k + ...]
    out = np.empty((B, T, C), np.float32)
    for b in range(B):
        ev = res.results[2 * b]["out"].astype(np.float32)
        od = res.results[2 * b + 1]["out"].astype(np.float32)
        for w in range(NW):
            o0, k = w * (WIN // 2), WIN // P
            for g, core_out in ((0, ev), (1, od)):
                blk = core_out[o0 : o0 + WIN // 2].reshape(64, k, C)
                dst = out[b, w * WIN : (w + 1) * WIN].reshape(k, 128, C)
                dst[:, 64 * g : 64 * g + 64, :] = blk.transpose(1, 0, 2)
    return out, res


def kernel(**inputs):
    out, _ = _run(inputs)
    return out
